# revision 3
# baseline (speedup 1.0000x reference)
"""Trainium2 Bass kernel for banded-cosine-similarity QA span logits.

Contract: kernel(**inputs) takes FULL inputs (sequence_outputs [8,2048,2048] f32,
idxs [8,2] int64) and returns the full output tuple (start_logits, end_logits),
each [8,2048] f32.  Sharding: pure data parallel, one example per NeuronCore.

Per-core computation (S=2048 rows, H=2048 hidden, band W=30):
  dot1 = seq @ q1, dot2 = seq @ q2, nsq = rowsum(seq^2)   (the memory-bound part)
  sim[i,w] = (dot1[i]+dot2[i+w]) / (qnorm*sqrt(nsq[i]+nsq[i+w]))  masked band
  start = rowmax, end = anti-diagonal scatter-max of the row-argmax, plus a
  mean/std sign-flip heuristic.

Engine split for the heavy reductions over the [2048,2048] f32 matrix:
  - ScalarE (ACT): nsq via activation(Square, accum_out)
  - VectorE (DVE): dot1/dot2 via fused scalar_tensor_tensor with accum_out
The band/argmax/scatter phases run on DVE/ACT/PE over [128, 16*30] tiles.

build_program(n_iters) can emit the whole input-dependent computation n_iters
times inside one NEFF so steady-state HW exec time can be measured without the
per-dispatch RPC overhead of the axon tunnel dominating.
"""

import os
import numpy as np
from contextlib import ExitStack

import concourse.bass as bass
import concourse.tile as tile
import concourse.bacc as bacc
from concourse import mybir, masks
from concourse.bass_utils import run_bass_kernel_spmd

f32 = mybir.dt.float32
AF = mybir.ActivationFunctionType
OP = mybir.AluOpType

B = 8
S = 2048
H = 2048
W = 30
P = 128
T = S // P          # 16 row tiles
C = H // P          # 16 h chunks
NEG = -1.0e30

KERN_STAGE = int(os.environ.get('KERN_STAGE', '99'))


def _emit_consts(tc, ctx):
    """Compile-time constants shared by all iterations."""
    nc = tc.nc
    cpool = ctx.enter_context(tc.tile_pool(name="consts", bufs=1))
    cst = {}
    ident = cpool.tile([P, P], f32)
    masks.make_identity(nc, ident[:])
    # bigI[k, y] = 1 iff y == k + W: slices give shifted identities
    bigI = cpool.tile([P, P + 2 * W + P], f32)
    nc.gpsimd.memset(bigI[:], 0.0)
    nc.gpsimd.affine_select(
        out=bigI[:], in_=bigI[:], compare_op=OP.not_equal, fill=1.0,
        base=W, channel_multiplier=1, pattern=[[-1, P + 2 * W + P]])
    ones = cpool.tile([P, 1], f32)
    nc.vector.memset(ones[:], 1.0)
    zeros16 = cpool.tile([P, T], f32)
    nc.vector.memset(zeros16[:], 0.0)
    negm001 = cpool.tile([P, T], f32)
    nc.vector.memset(negm001[:], -0.001)
    ninf_big = cpool.tile([P, T * W], f32)
    nc.vector.memset(ninf_big[:], NEG)
    zpad = cpool.tile([1, 32], f32)
    nc.vector.memset(zpad[:], 0.0)
    cst.update(ident=ident, bigI=bigI, ones=ones, zeros16=zeros16,
               negm001=negm001, ninf_big=ninf_big, zpad=zpad)
    return cst


def _emit_iter(tc, ctx, aps, cst, it):
    """One full evaluation of the per-example computation (input-dependent)."""
    nc = tc.nc
    seq_d = aps["seq"]
    qf_d = aps["qf"]
    qb_d = aps["qb"]
    mask_d = aps["maskadd"]
    rv_d = aps["rv"]
    out_d = aps["out"]
    d2f = aps[f"d2f{it}"]
    sc_d = aps[f"sc{it}"]
    scb_d = aps[f"scb{it}"]
    nsf = aps[f"nsf{it}"]

    ident = cst["ident"]; bigI = cst["bigI"]; ones = cst["ones"]
    zeros16 = cst["zeros16"]; negm001 = cst["negm001"]
    ninf_big = cst["ninf_big"]; zpad = cst["zpad"]

    persist = ctx.enter_context(tc.tile_pool(name=f"persist{it}", bufs=1))
    xpool = ctx.enter_context(tc.tile_pool(name=f"xpool{it}", bufs=3))
    scr_act_p = ctx.enter_context(tc.tile_pool(name=f"scr_act{it}", bufs=2))
    scr_dve_p = ctx.enter_context(tc.tile_pool(name=f"scr_dve{it}", bufs=2))
    pst_p = ctx.enter_context(tc.tile_pool(name=f"pst{it}", bufs=2, space="PSUM"))
    psh_p = ctx.enter_context(tc.tile_pool(name=f"psh{it}", bufs=4, space="PSUM"))

    qb_sb = persist.tile([P, 2 * C], f32)
    nc.sync.dma_start(qb_sb[:], qb_d[:])
    mask_sb = persist.tile([P, T * W], f32)
    nc.sync.dma_start(mask_sb[:], mask_d[:])
    rv_sb = persist.tile([P, T], mybir.dt.uint8)
    nc.sync.dma_start(rv_sb[:], rv_d[:])
    # HW DGE mishandles wide 0-step partition broadcasts from DRAM, so
    # replicate across partitions by doubling SBUF->SBUF DMAs instead.
    q12b = persist.tile([P, 2 * H], f32)
    nc.gpsimd.dma_start(q12b[0:1, :], qf_d[:].rearrange("a b -> (a b)").unsqueeze(0))
    k = 1
    while k < P:
        nc.gpsimd.dma_start(q12b[k:2 * k, :], q12b[0:k, :])
        k *= 2
    q1b = q12b[:, 0:H]
    q2b = q12b[:, H:2 * H]

    dot1_cols = persist.tile([P, T], f32)
    dot2_cols = persist.tile([P, T], f32)
    nsq_cols = persist.tile([P, T], f32)

    # ---- qnorm^2 ----
    qscr = persist.tile([P, 2 * C], f32)
    qcol = persist.tile([P, 1], f32)
    nc.scalar.activation(qscr[:], qb_sb[:], AF.Square, accum_out=qcol[:])
    ps_q = pst_p.tile([1, 1], f32, tag="ps_small")
    nc.tensor.matmul(ps_q[:], ones[:], qcol[:], start=True, stop=True)
    qn2_s = persist.tile([1, 1], f32)
    nc.vector.tensor_copy(qn2_s[:], ps_q[:])

    # SBUF partition-broadcast of a [1,1] scalar requires a DRAM bounce
    def bcast_scalar(s11, out_p1, slot):
        nc.sync.dma_start(sc_d[0:1, slot:slot + 1], s11[:])
        nc.sync.dma_start(out_p1[:], sc_d[0:1, slot:slot + 1].broadcast_to([P, 1]))

    qn2_b = persist.tile([P, 1], f32)
    bcast_scalar(qn2_s, qn2_b, 0)

    if KERN_STAGE < 2:
        return
    # ---- phase A: per row-tile reductions ----
    for t in range(T):
        x = xpool.tile([P, H], f32, tag="x")
        eng = nc.sync if t % 2 == 0 else nc.scalar
        eng.dma_start(x[:], seq_d[t * P:(t + 1) * P, :])

        # nsq on ACT
        sa = scr_act_p.tile([P, H], f32, tag="sa")
        nc.scalar.activation(sa[:], x[:], AF.Square,
                             accum_out=nsq_cols[:, t:t + 1])

        sv = scr_dve_p.tile([P, H], f32, tag="sv")
        nc.vector.scalar_tensor_tensor(
            out=sv[:], in0=x[:], scalar=1.0, in1=q1b,
            op0=OP.mult, op1=OP.mult, accum_out=dot1_cols[:, t:t + 1])
        sv2 = scr_dve_p.tile([P, H], f32, tag="sv")
        nc.vector.scalar_tensor_tensor(
            out=sv2[:], in0=x[:], scalar=1.0, in1=q2b,
            op0=OP.mult, op1=OP.mult, accum_out=dot2_cols[:, t:t + 1])

    if KERN_STAGE < 3:
        return
    # ---- phase B: flatten vectors to DRAM, band-gather back ----
    d2flat_w = bass.AP(d2f.tensor, 0, [[1, P], [P, T]])
    nc.sync.dma_start(d2flat_w, dot2_cols[:])
    nsflat_w = bass.AP(nsf.tensor, 0, [[1, P], [P, T]])
    nc.sync.dma_start(nsflat_w, nsq_cols[:])
    nc.sync.dma_start(bass.AP(d2f.tensor, S, [[32, 1], [1, 32]]), zpad[:])
    nc.sync.dma_start(bass.AP(nsf.tensor, S, [[32, 1], [1, 32]]), zpad[:])

    d2_all = persist.tile([P, T * W], f32)
    nc.sync.dma_start(
        d2_all[:].rearrange("p (t w) -> p t w", w=W),
        bass.AP(d2f.tensor, 0, [[1, P], [P, T], [1, W]]))
    n2_all = persist.tile([P, T * W], f32)
    nc.sync.dma_start(
        n2_all[:].rearrange("p (t w) -> p t w", w=W),
        bass.AP(nsf.tensor, 0, [[1, P], [P, T], [1, W]]))

    if KERN_STAGE < 4:
        return
    # ---- phase C: banded similarity, max, scatter-max ----
    d1v = dot1_cols[:].unsqueeze(2).broadcast_to([P, T, W])
    nsv = nsq_cols[:].unsqueeze(2).broadcast_to([P, T, W])

    s_all = persist.tile([P, T * W], f32)
    nc.vector.tensor_tensor(out=s_all[:].rearrange("p (t w) -> p t w", w=W),
                            in0=n2_all[:].rearrange("p (t w) -> p t w", w=W),
                            in1=nsv, op=OP.add)
    den = persist.tile([P, T * W], f32)
    nc.scalar.activation(den[:], s_all[:], AF.Sqrt, scale=qn2_b[:])
    num = persist.tile([P, T * W], f32)
    nc.vector.tensor_tensor(out=num[:].rearrange("p (t w) -> p t w", w=W),
                            in0=d2_all[:].rearrange("p (t w) -> p t w", w=W),
                            in1=d1v, op=OP.add)
    rden = persist.tile([P, T * W], f32)
    nc.vector.reciprocal(rden[:], den[:])
    simv = persist.tile([P, T * W], f32)
    nc.vector.tensor_tensor(out=simv[:], in0=num[:], in1=rden[:], op=OP.mult)
    simm = persist.tile([P, T * W], f32)
    nc.vector.tensor_tensor(out=simm[:], in0=simv[:], in1=mask_sb[:], op=OP.add)

    smax = persist.tile([P, T], f32)
    nc.vector.tensor_reduce(smax[:], simm[:].rearrange("p (t w) -> p t w", w=W),
                            axis=mybir.AxisListType.X, op=OP.max)

    if KERN_STAGE < 41:
        return
    eq = persist.tile([P, T * W], mybir.dt.uint8)
    nc.vector.tensor_tensor(out=eq[:].rearrange("p (t w) -> p t w", w=W),
                            in0=simm[:].rearrange("p (t w) -> p t w", w=W),
                            in1=smax[:].unsqueeze(2).broadcast_to([P, T, W]),
                            op=OP.is_equal)
    e_all = persist.tile([P, T * W], f32)
    nc.scalar.copy(e_all[:], ninf_big[:])
    nc.vector.copy_predicated(e_all[:], eq[:], simm[:])

    if KERN_STAGE < 42:
        return
    # anti-diagonal scatter-max via PE shifted identities:
    # D_w[p, t] = E[128t + p - w] ; endv = max_w D_w.  Shift-by-w =
    # matmul with bigI slices (exact 0/1 weights; E uses -1e30 not -inf
    # so 0 * E stays 0).  Fake 0s only reach rows e < W < sep0+1, where
    # endv has no real contribution and end_logits is 0 either way.
    e3 = e_all[:].rearrange("p (t w) -> p t w", w=W)
    endv = persist.tile([P, T], f32)
    nc.vector.memset(endv[:], NEG)
    for w in range(W):
        psh = psh_p.tile([P, T], f32, tag="psh")
        nc.tensor.matmul(psh[:], bigI[:, W - w:W - w + P], e3[:, :, w],
                         start=True, stop=(w == 0))
        if w > 0:
            nc.tensor.matmul(psh[:, 1:T], bigI[:, W - w + P:W - w + 2 * P],
                             e3[:, 0:T - 1, w], start=False, stop=True)
        nc.vector.tensor_tensor(out=endv[:], in0=endv[:], in1=psh[:],
                                op=OP.max)

    if KERN_STAGE < 43:
        return
    # end_logits = where(endv == -inf, 0, endv)
    eq2 = persist.tile([P, T], mybir.dt.uint8)
    nc.vector.tensor_tensor(out=eq2[:], in0=endv[:], in1=ninf_big[:, 0:T],
                            op=OP.is_equal)
    end_lg = persist.tile([P, T], f32)
    nc.vector.select(end_lg[:], eq2[:], zeros16[:], endv[:])
    # start_logits = where(row_valid, smax, 0)
    start_lg = persist.tile([P, T], f32)
    nc.vector.select(start_lg[:], rv_sb[:], smax[:], zeros16[:])

    if KERN_STAGE < 6:
        return
    # ---- phase D: stats + flip ----
    stat_row = persist.tile([1, P], f32)

    def cross_max(x16, out11, tagsfx):
        colmax = persist.tile([P, 1], f32, tag="colmax" + tagsfx)
        nc.vector.tensor_reduce(colmax[:], x16[:], axis=mybir.AxisListType.X,
                                op=OP.max)
        nc.sync.dma_start(stat_row[:], colmax[:])
        nc.vector.tensor_reduce(out11[:], stat_row[:],
                                axis=mybir.AxisListType.X, op=OP.max)

    def mean_std(x16, tagsfx):
        colsum = persist.tile([P, 1], f32, tag="cs" + tagsfx)
        nc.vector.tensor_reduce(colsum[:], x16[:], axis=mybir.AxisListType.X,
                                op=OP.add)
        ps = pst_p.tile([1, 1], f32, tag="ps_small")
        nc.tensor.matmul(ps[:], ones[:], colsum[:], start=True, stop=True)
        m = persist.tile([1, 1], f32, tag="m" + tagsfx)
        nc.scalar.mul(m[:], ps[:], 1.0 / S)
        negm = persist.tile([1, 1], f32, tag="nm" + tagsfx)
        nc.scalar.mul(negm[:], m[:], -1.0)
        negm_b = persist.tile([P, 1], f32, tag="nmb" + tagsfx)
        bcast_scalar(negm, negm_b, 1 if tagsfx == "s" else 2)
        scr = persist.tile([P, T], f32, tag="scr" + tagsfx)
        sqcol = persist.tile([P, 1], f32, tag="sq" + tagsfx)
        nc.scalar.activation(scr[:], x16[:], AF.Square, bias=negm_b[:],
                             accum_out=sqcol[:])
        ps2 = pst_p.tile([1, 1], f32, tag="ps_small")
        nc.tensor.matmul(ps2[:], ones[:], sqcol[:], start=True, stop=True)
        var = persist.tile([1, 1], f32, tag="v" + tagsfx)
        nc.scalar.mul(var[:], ps2[:], 1.0 / (S - 1))
        sd = persist.tile([1, 1], f32, tag="sd" + tagsfx)
        nc.scalar.activation(sd[:], var[:], AF.Sqrt)
        thr = persist.tile([1, 1], f32, tag="thr" + tagsfx)
        nc.vector.tensor_tensor(out=thr[:], in0=m[:], in1=sd[:], op=OP.add)
        return thr

    maxs = persist.tile([1, 1], f32)
    cross_max(start_lg, maxs, "s")
    thr_s = mean_std(start_lg, "s")
    thr_e = mean_std(end_lg, "e")
    fl_s = persist.tile([1, 1], mybir.dt.uint8)
    nc.vector.tensor_tensor(out=fl_s[:], in0=maxs[:], in1=thr_s[:], op=OP.is_lt)
    fl_e = persist.tile([1, 1], mybir.dt.uint8)
    nc.vector.tensor_tensor(out=fl_e[:], in0=maxs[:], in1=thr_e[:], op=OP.is_lt)
    flip = persist.tile([1, 1], mybir.dt.uint8)
    nc.vector.tensor_tensor(out=flip[:], in0=fl_s[:], in1=fl_e[:], op=OP.max)
    flip_b = persist.tile([P, 1], mybir.dt.uint8)
    nc.sync.dma_start(scb_d[0:1, 0:1], flip[:])
    nc.sync.dma_start(flip_b[:], scb_d[0:1, 0:1].broadcast_to([P, 1]))

    if KERN_STAGE < 7:
        return
    # ---- phase E: apply flip, write outputs ----
    for k, x16 in enumerate((start_lg, end_lg)):
        negx = persist.tile([P, T], f32, tag=f"negx{k}")
        nc.vector.tensor_scalar_mul(negx[:], x16[:], -1.0)
        isz = persist.tile([P, T], mybir.dt.uint8, tag=f"isz{k}")
        nc.vector.tensor_tensor(out=isz[:], in0=x16[:], in1=zeros16[:],
                                op=OP.is_equal)
        negged = persist.tile([P, T], f32, tag=f"ngd{k}")
        nc.vector.select(negged[:], isz[:], negm001[:], negx[:])
        outv = persist.tile([P, T], f32, tag=f"outv{k}")
        nc.vector.select(outv[:], flip_b[:].broadcast_to([P, T]), negged[:],
                         x16[:])
        nc.sync.dma_start(bass.AP(out_d.tensor, k * S, [[1, P], [P, T]]),
                          outv[:])


_NC_CACHE = {}


def build_program(n_iters=1):
    key = (n_iters, KERN_STAGE)
    if key in _NC_CACHE:
        return _NC_CACHE[key]
    nc = bacc.Bacc("TRN2", target_bir_lowering=False, debug=False)
    aps = {
        "seq": nc.dram_tensor("seq", [S, H], f32, kind="ExternalInput").ap(),
        "qf": nc.dram_tensor("qf", [2, H], f32, kind="ExternalInput").ap(),
        "qb": nc.dram_tensor("qb", [P, 2 * C], f32, kind="ExternalInput").ap(),
        "maskadd": nc.dram_tensor("maskadd", [P, T * W], f32,
                                  kind="ExternalInput").ap(),
        "rv": nc.dram_tensor("rv", [P, T], mybir.dt.uint8,
                             kind="ExternalInput").ap(),
        "out": nc.dram_tensor("out", [2, S], f32, kind="ExternalOutput").ap(),
    }
    for it in range(n_iters):
        aps[f"d2f{it}"] = nc.dram_tensor(f"d2f{it}", [S + 32], f32).ap()
        aps[f"nsf{it}"] = nc.dram_tensor(f"nsf{it}", [S + 32], f32).ap()
        aps[f"sc{it}"] = nc.dram_tensor(f"sc{it}", [1, 8], f32).ap()
        aps[f"scb{it}"] = nc.dram_tensor(f"scb{it}", [1, 8],
                                         mybir.dt.uint8).ap()
    with tile.TileContext(nc) as tc, ExitStack() as cctx:
        cst = _emit_consts(tc, cctx)
        for it in range(n_iters):
            with ExitStack() as ctx:
                _emit_iter(tc, ctx, aps, cst, it)
    nc.compile()
    _NC_CACHE[key] = nc
    return nc


def host_prep(seq, idx):
    """Per-core derived inputs from one example. seq [S,H] f32, idx [2] int."""
    sep0, sep1 = int(idx[0]), int(idx[1])
    q1 = np.ascontiguousarray(seq[1])
    q2 = np.ascontiguousarray(seq[sep0 - 1])
    qf = np.stack([q1, q2])                                    # [2,H]
    qb = np.empty((P, 2 * C), np.float32)
    qb[:, 0::2] = q1.reshape(C, P).T
    qb[:, 1::2] = q2.reshape(C, P).T
    i = np.arange(S)[:, None]                                  # [S,1]
    w = np.arange(W)[None, :]
    valid = (i >= sep0 + 1) & (i < sep1) & ((i + w) < sep1)    # [S,W]
    maskadd = np.where(valid, np.float32(0), np.float32(NEG))
    # [S,W] -> [P, T*W] with row r=(128t+p) at [p, t*W+w]
    maskadd = np.ascontiguousarray(
        maskadd.reshape(T, P, W).transpose(1, 0, 2).reshape(P, T * W))
    rv = ((np.arange(S) >= sep0 + 1) & (np.arange(S) < sep1)).astype(np.uint8)
    rv = np.ascontiguousarray(rv.reshape(T, P).T)
    return {"seq": seq, "qf": qf, "qb": qb, "maskadd": maskadd, "rv": rv}


# ---------------------------------------------------------------------------
# PJRT runner: same execution path run_bass_kernel_spmd takes under axon
# (bass2jax custom-call -> NEFF via PJRT), but shards per-device without the
# host-side 128MB concat, and caches the jitted executable across calls.
# ---------------------------------------------------------------------------
_RUNNER_CACHE = {}


def _make_runner(nc, n_cores):
    key = id(nc)
    if key in _RUNNER_CACHE:
        return _RUNNER_CACHE[key]
    import jax
    import warnings
    from jax.sharding import Mesh, PartitionSpec, NamedSharding
    with warnings.catch_warnings():
        warnings.simplefilter("ignore")
        from jax.experimental.shard_map import shard_map
    from concourse.bass2jax import (_bass_exec_p, install_neuronx_cc_hook,
                                    partition_id_tensor)
    install_neuronx_cc_hook()

    partition_name = (nc.partition_id_tensor.name
                      if nc.partition_id_tensor else None)
    in_names, out_names, out_avals, out_zero_shapes = [], [], [], []
    for alloc in nc.m.functions[0].allocations:
        if not isinstance(alloc, mybir.MemoryLocationSet):
            continue
        name = alloc.memorylocations[0].name
        if alloc.kind == "ExternalInput":
            if name != partition_name:
                in_names.append(name)
        elif alloc.kind == "ExternalOutput":
            shape = tuple(alloc.tensor_shape)
            dtype = mybir.dt.np(alloc.dtype)
            out_names.append(name)
            out_avals.append(jax.core.ShapedArray(shape, dtype))
            out_zero_shapes.append((shape, dtype))
    n_params = len(in_names)
    n_outs = len(out_avals)
    in_names_all = in_names + out_names + (
        [partition_name] if partition_name else [])

    def _body(*args):
        operands = list(args)
        if partition_name is not None:
            operands.append(partition_id_tensor())
        outs = _bass_exec_p.bind(
            *operands, out_avals=tuple(out_avals),
            in_names=tuple(in_names_all), out_names=tuple(out_names),
            lowering_input_output_aliases=(), sim_require_finite=True,
            sim_require_nnan=True, nc=nc)
        return tuple(outs)

    devices = jax.devices()[:n_cores]
    mesh = Mesh(np.asarray(devices), ("core",))
    sh = NamedSharding(mesh, PartitionSpec("core"))
    donate = tuple(range(n_params, n_params + n_outs))
    fn = jax.jit(
        shard_map(_body, mesh=mesh,
                  in_specs=(PartitionSpec("core",),) * (n_params + n_outs),
                  out_specs=(PartitionSpec("core",),) * n_outs,
                  check_rep=False),
        donate_argnums=donate, keep_unused=True)
    runner = dict(fn=fn, in_names=in_names, out_names=out_names,
                  out_avals=out_avals, out_zero_shapes=out_zero_shapes,
                  mesh=mesh, sharding=sh, devices=devices, n_cores=n_cores)
    _RUNNER_CACHE[key] = runner
    return runner


def _stage_inputs(runner, in_maps):
    """device_put each per-core input straight to its device (no host concat)."""
    import jax
    n = runner["n_cores"]
    devices = runner["devices"]
    staged = []
    for name in runner["in_names"]:
        shards = [jax.device_put(np.asarray(in_maps[c][name]), devices[c])
                  for c in range(n)]
        s0 = shards[0]
        global_shape = (n * s0.shape[0],) + tuple(s0.shape[1:])
        arr = jax.make_array_from_single_device_arrays(
            global_shape, runner["sharding"], shards)
        staged.append(arr)
    return staged


def _stage_zero_outs(runner):
    import jax
    n = runner["n_cores"]
    zs = []
    for shape, dtype in runner["out_zero_shapes"]:
        z = np.zeros((n * shape[0],) + tuple(shape[1:]), dtype)
        zs.append(jax.device_put(z, runner["sharding"]))
    return zs


def run_pjrt(nc, in_maps, n_cores=B):
    runner = _make_runner(nc, n_cores)
    staged = _stage_inputs(runner, in_maps)
    zouts = _stage_zero_outs(runner)
    out_arrs = runner["fn"](*staged, *zouts)
    res = []
    for c in range(n_cores):
        res.append({
            name: np.asarray(out_arrs[i]).reshape(
                n_cores, *runner["out_avals"][i].shape)[c]
            for i, name in enumerate(runner["out_names"])})
    return res


def kernel(sequence_outputs, idxs):
    sequence_outputs = np.asarray(sequence_outputs, dtype=np.float32)
    idxs = np.asarray(idxs)
    nc = build_program()
    in_maps = [host_prep(sequence_outputs[c], idxs[c]) for c in range(B)]
    try:
        results = run_pjrt(nc, in_maps)
    except Exception:
        results = run_bass_kernel_spmd(
            nc, in_maps, core_ids=list(range(B))).results
    outs = np.stack([results[c]["out"] for c in range(B)])  # [B,2,S]
    start = np.ascontiguousarray(outs[:, 0, :])
    end = np.ascontiguousarray(outs[:, 1, :])
    return start, end
